# revision 1
# baseline (speedup 1.0000x reference)
"""ODE-RNN (nn_ODERNN) Trainium2 Bass kernel.

Strategy: data-parallel over batch across 8 NeuronCores (4 batches x 34
particle-slots = 136 independent rows per core, processed as G row-groups
whose independent recurrence chains pipeline across engines). State kept
transposed [DRNN=128 partitions, rows on free dim]. All matmuls in fp32 —
the recurrence amplifies rounding noise ~1.14x/step over 128 steps, so any
bf16/tf32 rounding in the loop destroys accuracy (measured). ODE Euler
steps telescoped through PSUM accumulation:
    z1_{k+1} = z1_k + (DT*w2@w0)^T a2_k    (accumulate in PSUM)
    h1       = h0 + DT*w2^T sum_k(a2_k)    (single fp32 update per step)
PSUM bank invariant (verified on HW): at most ONE open matmul accumulation
group per bank at a time; gi/gh gate pairs open+close back-to-back per
bank region. The mask update is folded as h_next = (1-g) h1 + g n1 with
g = m*(1-z1), and the output projection runs per step from PSUM.
"""

import os
import numpy as np
import ml_dtypes

B, S, P, J = 32, 128, 2, 17
G = int(os.environ.get("ODERNN_G", "2"))
DIN, DOUT, DRNN, DHID = 2, 3, 128, 256
N = P * J            # 34
DT = 0.1
K = 4                # Euler steps
NCORES = 8
BL = B // NCORES     # 4 batches per core
R = BL * N           # 136 rows per core

# dev override: fewer scan steps (harness always uses full S)
NSTEPS = int(os.environ.get("ODERNN_STEPS", S))
SR = NSTEPS * R

BF16 = ml_dtypes.bfloat16

_prog_cache = {}


def _build_program(flags):
    """Trace the Bass/Tile program. `flags` is a tuple of bools for which
    bias groups are nonzero (affects emitted ops)."""
    import concourse.bass as bass
    import concourse.tile as tile
    import concourse.mybir as mybir
    from concourse import bacc

    (fb01, fb2, frz0, fg0h, frz1, fg1h, fbout) = flags

    dt = mybir.dt
    f32 = dt.float32
    bf16 = dt.bfloat16
    Alu = mybir.AluOpType
    Act = mybir.ActivationFunctionType

    nc = bacc.Bacc("TRN2", target_bir_lowering=False)

    # ---- DRAM I/O ----
    d_xm = nc.dram_tensor("xm", [DIN, SR], f32, kind="ExternalInput")
    d_mbc = nc.dram_tensor("mbc", [DRNN, SR], bf16, kind="ExternalInput")
    d_h0f = nc.dram_tensor("h0f", [DRNN, R], f32, kind="ExternalInput")
    d_w0 = nc.dram_tensor("w0", [DRNN, DHID], f32, kind="ExternalInput")
    d_w1a = nc.dram_tensor("w1a", [128, DHID], f32, kind="ExternalInput")
    d_w1b = nc.dram_tensor("w1b", [128, DHID], f32, kind="ExternalInput")
    d_w2a = nc.dram_tensor("w2a", [128, DRNN], f32, kind="ExternalInput")
    d_w2b = nc.dram_tensor("w2b", [128, DRNN], f32, kind="ExternalInput")
    d_W20a = nc.dram_tensor("W20a", [128, DHID], f32, kind="ExternalInput")
    d_W20b = nc.dram_tensor("W20b", [128, DHID], f32, kind="ExternalInput")
    d_wih0 = nc.dram_tensor("wih0", [DIN, 3 * DRNN], f32, kind="ExternalInput")
    d_whh0 = nc.dram_tensor("whh0", [DRNN, 3 * DRNN], f32, kind="ExternalInput")
    d_wih1 = nc.dram_tensor("wih1", [DRNN, 3 * DRNN], f32, kind="ExternalInput")
    d_whh1 = nc.dram_tensor("whh1", [DRNN, 3 * DRNN], f32, kind="ExternalInput")
    d_wout = nc.dram_tensor("wout", [DRNN, DOUT], f32, kind="ExternalInput")
    d_bias = nc.dram_tensor("biaspk", [DRNN, 24], f32, kind="ExternalInput")
    d_y = nc.dram_tensor("y", [NSTEPS, DOUT, R], f32, kind="ExternalOutput")

    DBG = os.environ.get("ODERNN_DEBUG", "0") == "1"
    dbg_tensors = {}

    def dbg_out(name, ap, shape, dtype):
        if not DBG:
            return
        dten = nc.dram_tensor("dbg_" + name, list(shape), dtype,
                              kind="ExternalOutput")
        dbg_tensors[name] = dten
        nc.sync.dma_start(out=dten[:], in_=ap)

    with tile.TileContext(nc) as tc:
        wp = tc.alloc_tile_pool(name="wconst", bufs=1)
        st = tc.alloc_tile_pool(name="state", bufs=6)
        wk = tc.alloc_tile_pool(name="work", bufs=4)
        pp = tc.alloc_tile_pool(name="psum", bufs=1, space="PSUM")

        def load(pool, dram, shape, dtype, name):
            t = pool.tile(shape, dtype, tag=name, name=name)
            nc.sync.dma_start(out=t[:], in_=dram[:])
            return t

        xm = load(wp, d_xm, [DIN, SR], f32, "xm")
        mbc = load(wp, d_mbc, [DRNN, SR], bf16, "mbc")
        h0f = load(wp, d_h0f, [DRNN, R], f32, "h0f")
        w0 = load(wp, d_w0, [DRNN, DHID], f32, "w0")
        w1a = load(wp, d_w1a, [128, DHID], f32, "w1a")
        w1b = load(wp, d_w1b, [128, DHID], f32, "w1b")
        w2a = load(wp, d_w2a, [128, DRNN], f32, "w2a")
        w2b = load(wp, d_w2b, [128, DRNN], f32, "w2b")
        W20a = load(wp, d_W20a, [128, DHID], f32, "W20a")
        W20b = load(wp, d_W20b, [128, DHID], f32, "W20b")
        wih0 = load(wp, d_wih0, [DIN, 3 * DRNN], f32, "wih0")
        whh0 = load(wp, d_whh0, [DRNN, 3 * DRNN], f32, "whh0")
        wih1 = load(wp, d_wih1, [DRNN, 3 * DRNN], f32, "wih1")
        whh1 = load(wp, d_whh1, [DRNN, 3 * DRNN], f32, "whh1")
        wout = load(wp, d_wout, [DRNN, DOUT], f32, "wout")
        bias = load(wp, d_bias, [DRNN, 24], f32, "biaspk")

        MM = nc.tensor.matmul
        ACT = nc.scalar.activation
        V = nc.vector

        RG = R // G
        hprev = [h0f[:, gi * RG:(gi + 1) * RG] for gi in range(G)]

        for s in range(NSTEPS):
          for gi in range(G):
            c0 = s * R + gi * RG
            c1 = c0 + RG
            xsl = xm[:, c0:c1]
            msl = mbc[:, c0:c1]
            hprev_f = hprev[gi]

            # PSUM bank invariant: at most ONE open accumulation group per
            # bank at any time (two interleaved open groups in one bank get
            # corrupted by intervening matmul groups — verified on HW).
            z1a = pp.tile([128, RG], f32, tag="z1a", name="z1a")
            z1b = pp.tile([128, RG], f32, tag="z1b", name="z1b")
            z2 = pp.tile([128, 2, RG], f32, tag="z2", name="z2")
            hd = pp.tile([128, RG], f32, tag="hd", name="hd")
            prz0 = pp.tile([128, 2, RG], f32, tag="prz0", name="prz0")
            pg0 = pp.tile([128, 2, RG], f32, tag="pg0", name="pg0")
            prz1 = pp.tile([128, 2, RG], f32, tag="prz1", name="prz1")
            pg1 = pp.tile([128, 2, RG], f32, tag="pg1", name="pg1")

            # ---- ODE: z1_0 = w0^T h_prev (open accumulation groups,
            # one per bank, telescoped across Euler steps) ----
            MM(z1a[:], w0[:, 0:128], hprev_f[:], start=True, stop=False)
            MM(z1b[:], w0[:, 128:256], hprev_f[:], start=True, stop=False)

            # gi0_g = wih0_g^T x (closed group; its own bank region)
            MM(pg0[:, 0, :], wih0[:, 256:384], xsl, start=True, stop=True)

            for k in range(K):
                last = k == K - 1
                a1 = wk.tile([128, 2, RG], f32, tag="a1")
                if not fb01:
                    ACT(a1[:, 0, :], z1a[:], Act.Tanh)
                    ACT(a1[:, 1, :], z1b[:], Act.Tanh)
                else:
                    # per-block bias: b0_blk + k*DT*(b2 @ w0)_blk
                    ACT(a1[:, 0, :], z1a[:], Act.Tanh,
                        bias=bias[:, 16 + 2 * k:17 + 2 * k])
                    ACT(a1[:, 1, :], z1b[:], Act.Tanh,
                        bias=bias[:, 17 + 2 * k:18 + 2 * k])
                MM(z2[:, 0, :], w1a[:, 0:128], a1[:, 0, :], start=True, stop=False)
                MM(z2[:, 0, :], w1b[:, 0:128], a1[:, 1, :], start=False, stop=True)
                MM(z2[:, 1, :], w1a[:, 128:256], a1[:, 0, :], start=True, stop=False)
                MM(z2[:, 1, :], w1b[:, 128:256], a1[:, 1, :], start=False, stop=True)
                a2 = wk.tile([128, 2, RG], f32, tag="a2")
                if not fb01:
                    ACT(a2[:], z2[:], Act.Tanh)
                else:
                    ACT(a2[:, 0, :], z2[:, 0, :], Act.Tanh, bias=bias[:, 2:3])
                    ACT(a2[:, 1, :], z2[:, 1, :], Act.Tanh, bias=bias[:, 3:4])
                if not last:
                    MM(z1a[:], W20a[:, 0:128], a2[:, 0, :], start=False, stop=False)
                    MM(z1a[:], W20b[:, 0:128], a2[:, 1, :], start=False, stop=(k == K - 2))
                    MM(z1b[:], W20a[:, 128:256], a2[:, 0, :], start=False, stop=False)
                    MM(z1b[:], W20b[:, 128:256], a2[:, 1, :], start=False, stop=(k == K - 2))
                MM(hd[:], w2a[:], a2[:, 0, :], start=(k == 0), stop=False)
                MM(hd[:], w2b[:], a2[:, 1, :], start=False, stop=last)

            # h1 = h_prev + (hd + DT*b2)
            h1f = st.tile([128, RG], f32, tag="h1f")
            V.scalar_tensor_tensor(h1f[:], hd[:], bias[:, 4:5], hprev_f[:],
                                   op0=Alu.add, op1=Alu.add)
            h1b = h1f[:]

            # ---- GRU0 ----
            # gi/gh pairs per bank region, opened and closed back-to-back
            MM(prz0[:, 0, :], wih0[:, 0:128], xsl, start=True, stop=False)
            MM(prz0[:, 0, :], whh0[:, 0:128], h1b, start=False, stop=True)
            MM(prz0[:, 1, :], wih0[:, 128:256], xsl, start=True, stop=False)
            MM(prz0[:, 1, :], whh0[:, 128:256], h1b, start=False, stop=True)
            MM(pg0[:, 1, :], whh0[:, 256:384], h1b, start=True, stop=True)

            if DBG and s == 0:
                prz0c = wk.tile([128, 2, RG], f32, tag="prz0c")
                V.tensor_copy(prz0c[:], prz0[:])
                dbg_out("prz0c", prz0c[:], [128, 2, RG], f32)
                pg0c = wk.tile([128, 2, RG], f32, tag="pg0c")
                V.tensor_copy(pg0c[:], pg0[:])
                dbg_out("pg0c", pg0c[:], [128, 2, RG], f32)

            rz0 = wk.tile([128, 2, RG], f32, tag="rz0")
            if not frz0:
                ACT(rz0[:], prz0[:], Act.Sigmoid)
            else:
                ACT(rz0[:, 0, :], prz0[:, 0, :], Act.Sigmoid, bias=bias[:, 5:6])
                ACT(rz0[:, 1, :], prz0[:, 1, :], Act.Sigmoid, bias=bias[:, 6:7])
            r0, zz0 = rz0[:, 0, :], rz0[:, 1, :]

            # gh1_g (hidden side of GRU1 n-gate; closed group, own region)
            MM(pg1[:, 1, :], whh1[:, 256:384], h1b, start=True, stop=True)

            # n0 = tanh(ig0 + r0*hg0 (+bg0)) — chain DVE ops first
            hg0 = pg0[:, 1, :]
            if fg0h:
                hg0t = wk.tile([128, RG], f32, tag="hg0t")
                nc.scalar.add(hg0t[:], hg0, bias[:, 8:9])
                hg0 = hg0t[:]
            s1 = wk.tile([128, RG], f32, tag="s1")
            V.tensor_mul(s1[:], r0, hg0)
            np0 = wk.tile([128, RG], f32, tag="np0")
            V.tensor_add(np0[:], s1[:], pg0[:, 0, :])
            n0 = wk.tile([128, RG], f32, tag="n0")
            ACT(n0[:], np0[:], Act.Tanh, bias=bias[:, 7:8])

            # off-chain helpers (overlap n0 on DVE)
            u0 = wk.tile([128, RG], f32, tag="u0")
            V.tensor_scalar(u0[:], zz0, -1.0, 1.0, op0=Alu.mult, op1=Alu.add)
            t0 = wk.tile([128, RG], f32, tag="t0")
            V.tensor_mul(t0[:], zz0, h1f[:])

            v0 = wk.tile([128, RG], f32, tag="v0")
            V.tensor_mul(v0[:], u0[:], n0[:])
            h2b = wk.tile([128, RG], f32, tag="h2b")
            V.tensor_add(h2b[:], v0[:], t0[:])

            # ---- GRU1 ---- (gh/gi pairs opened+closed back-to-back per region)
            MM(prz1[:, 0, :], whh1[:, 0:128], h1b, start=True, stop=False)
            MM(prz1[:, 0, :], wih1[:, 0:128], h2b[:], start=False, stop=True)
            MM(prz1[:, 1, :], whh1[:, 128:256], h1b, start=True, stop=False)
            MM(prz1[:, 1, :], wih1[:, 128:256], h2b[:], start=False, stop=True)
            MM(pg1[:, 0, :], wih1[:, 256:384], h2b[:], start=True, stop=True)

            rz1 = wk.tile([128, 2, RG], f32, tag="rz1")
            if not frz1:
                ACT(rz1[:], prz1[:], Act.Sigmoid)
            else:
                ACT(rz1[:, 0, :], prz1[:, 0, :], Act.Sigmoid, bias=bias[:, 9:10])
                ACT(rz1[:, 1, :], prz1[:, 1, :], Act.Sigmoid, bias=bias[:, 10:11])
            r1, zz1 = rz1[:, 0, :], rz1[:, 1, :]

            hg1 = pg1[:, 1, :]
            if fg1h:
                hg1t = wk.tile([128, RG], f32, tag="hg1t")
                nc.scalar.add(hg1t[:], hg1, bias[:, 12:13])
                hg1 = hg1t[:]
            s2 = wk.tile([128, RG], f32, tag="s2")
            V.tensor_mul(s2[:], r1, hg1)
            np1 = wk.tile([128, RG], f32, tag="np1")
            V.tensor_add(np1[:], s2[:], pg1[:, 0, :])
            n1 = wk.tile([128, RG], f32, tag="n1")
            ACT(n1[:], np1[:], Act.Tanh, bias=bias[:, 11:12])

            # g = m * (1 - z1); h_next = (1-g) h1 + g n1 (overlap n1 on DVE)
            u1 = wk.tile([128, RG], f32, tag="u1")
            V.tensor_scalar(u1[:], zz1, -1.0, 1.0, op0=Alu.mult, op1=Alu.add)
            g = wk.tile([128, RG], f32, tag="g")
            V.tensor_mul(g[:], u1[:], msl)
            tg = wk.tile([128, RG], f32, tag="tg")
            V.tensor_mul(tg[:], g[:], h1f[:])
            hm = wk.tile([128, RG], f32, tag="hm")
            V.tensor_sub(hm[:], h1f[:], tg[:])

            vg = wk.tile([128, RG], f32, tag="vg")
            V.tensor_mul(vg[:], g[:], n1[:])
            hn = st.tile([128, RG], f32, tag="hn")
            V.tensor_add(hn[:], vg[:], hm[:])

            # per-step output projection y_s = wout^T h1 (+bout); fully off
            # the critical chain — emitted last so it backfills idle slots
            py = pp.tile([DOUT, RG], f32, tag="hd", name="py")
            MM(py[:], wout[:], h1f[:], start=True, stop=True)
            ysl = wk.tile([DOUT, RG], f32, tag="ysl")
            if not fbout:
                V.tensor_copy(ysl[:], py[:])
            else:
                ACT(ysl[:], py[:], Act.Identity, bias=bias[0:DOUT, 15:16])
            nc.sync.dma_start(out=d_y[s, :, gi * RG:(gi + 1) * RG], in_=ysl[:])

            if s == 0:
                names = os.environ.get("ODERNN_DBG_NAMES",
                                       "h1f,rz0,n0,h2b,rz1,n1,hn").split(",")
                avail = {
                    "h1f": (h1f[:], [128, RG], f32),
                    "rz0": (rz0[:], [128, 2, RG], f32),
                    "u0": (u0[:], [128, RG], f32),
                    "t0": (t0[:], [128, RG], f32),
                    "s1": (s1[:], [128, RG], f32),
                    "np0": (np0[:], [128, RG], f32),
                    "n0": (n0[:], [128, RG], f32),
                    "h2b": (h2b[:], [128, RG], f32),
                    "rz1": (rz1[:], [128, 2, RG], f32),
                    "u1": (u1[:], [128, RG], f32),
                    "g": (g[:], [128, RG], f32),
                    "tg": (tg[:], [128, RG], f32),
                    "hm": (hm[:], [128, RG], f32),
                    "s2": (s2[:], [128, RG], f32),
                    "np1": (np1[:], [128, RG], f32),
                    "n1": (n1[:], [128, RG], f32),
                    "vg": (vg[:], [128, RG], f32),
                    "hn": (hn[:], [128, RG], f32),
                }
                for nm in names:
                    if nm in avail:
                        ap, shp, dty = avail[nm]
                        dbg_out(nm, ap, shp, dty)

            hprev[gi] = hn

        pp.release()
        wk.release()
        st.release()
        wp.release()

    nc.compile()
    return nc


def _to_bf(x):
    return np.ascontiguousarray(x.astype(BF16))


def _prep(inputs):
    """Host-side prep: shard over batch, transpose layouts, pack weights."""
    x2d = np.asarray(inputs["x2d"], np.float32)
    mask = np.asarray(inputs["mask"])
    g = lambda n: np.asarray(inputs[n], np.float32)
    w0, b0 = g("ode_w0"), g("ode_b0")
    w1, b1 = g("ode_w1"), g("ode_b1")
    w2, b2 = g("ode_w2"), g("ode_b2")
    wih0, whh0 = g("wih0"), g("whh0")
    bih0, bhh0 = g("bih0"), g("bhh0")
    wih1, whh1 = g("wih1"), g("whh1")
    bih1, bhh1 = g("bih1"), g("bhh1")
    wout, bout = g("wout"), g("bout")
    h0 = g("h0")

    mf = mask.astype(np.float32)
    xs = (x2d * mf).reshape(B, S, N, DIN)[:, :NSTEPS]
    ms = mf.reshape(B, S, N)[:, :NSTEPS]

    W20 = (DT * (w2.astype(np.float64) @ w0.astype(np.float64))).astype(np.float32)

    h0T = np.repeat(h0.reshape(DRNN, 1), R, axis=1).astype(np.float32)

    # bias pack [128, 16] (+ [128,8] tail for telescoped tanh biases when fb01)
    bp = np.zeros((DRNN, 24), np.float32)
    bp[:, 0], bp[:, 1] = b0[0:128], b0[128:256]
    bp[:, 2], bp[:, 3] = b1[0:128], b1[128:256]
    bp[:, 4] = DT * b2
    brz0 = bih0 + bhh0
    bp[:, 5], bp[:, 6] = brz0[0:128], brz0[128:256]
    bp[:, 7] = bih0[256:384]
    bp[:, 8] = bhh0[256:384]
    brz1 = bih1 + bhh1
    bp[:, 9], bp[:, 10] = brz1[0:128], brz1[128:256]
    bp[:, 11] = bih1[256:384]
    bp[:, 12] = bhh1[256:384]
    bp[0:DOUT, 15] = bout
    # telescoped z1 tanh bias terms: b0_blk + k*DT*(w0^T ... ) -- only used
    # when fb01; w0: (DRNN,DHID) so correction = DT * w0.T-like term of b2
    # propagated through h: z1 = w0^T h, h gains DT*b2 per Euler step =>
    # z1 bias gain per step = DT * (b2 @ w0)  (b2: [DRNN], w0: [DRNN,DHID])
    zb = DT * (b2 @ w0)  # [DHID]
    for k in range(K):
        bp[:, 16 + 2 * k + 0] = b0[0:128] + k * zb[0:128]
        bp[:, 16 + 2 * k + 1] = b0[128:256] + k * zb[128:256]

    flags = (
        bool(np.any(b0) or np.any(b1) or np.any(b2)),
        bool(np.any(b2)),
        bool(np.any(brz0[0:256])),
        bool(np.any(bhh0[256:384])),
        bool(np.any(brz1[0:256])),
        bool(np.any(bhh1[256:384])),
        bool(np.any(bout)),
    )

    C = np.ascontiguousarray
    shared = {
        "h0f": h0T,
        "w0": C(w0),
        "w1a": C(w1[0:128]),
        "w1b": C(w1[128:256]),
        "w2a": C(DT * w2[0:128]),
        "w2b": C(DT * w2[128:256]),
        "W20a": C(W20[0:128]),
        "W20b": C(W20[128:256]),
        "wih0": C(wih0),
        "whh0": C(whh0),
        "wih1": C(wih1),
        "whh1": C(whh1),
        "wout": C(wout),
        "biaspk": bp,
    }

    in_maps = []
    for c in range(NCORES):
        xc = xs[c * BL:(c + 1) * BL]           # (BL, NS, N, DIN)
        xmT = xc.transpose(3, 1, 0, 2).reshape(DIN, SR)
        mc = ms[c * BL:(c + 1) * BL]           # (BL, NS, N)
        mrow = mc.transpose(1, 0, 2).reshape(1, SR)
        mbc = np.broadcast_to(mrow, (DRNN, SR))
        m = dict(shared)
        m["xm"] = np.ascontiguousarray(xmT, np.float32)
        m["mbc"] = _to_bf(mbc)
        in_maps.append(m)
    return in_maps, flags


def kernel(**inputs):
    in_maps, flags = _prep(inputs)
    if flags not in _prog_cache:
        _prog_cache[flags] = _build_program(flags)
    nc = _prog_cache[flags]

    from concourse.bass_utils import run_bass_kernel_spmd
    res = run_bass_kernel_spmd(nc, in_maps, core_ids=list(range(NCORES)))
    global _last_results
    _last_results = res.results

    ys = np.zeros((B, NSTEPS, P, J, DOUT), np.float32)
    for c in range(NCORES):
        y = res.results[c]["y"]                      # (NSTEPS, DOUT, R)
        y = y.reshape(NSTEPS, DOUT, BL, N).transpose(2, 0, 3, 1)
        ys[c * BL:(c + 1) * BL] = y.reshape(BL, NSTEPS, P, J, DOUT)
    return ys



# revision 3
# speedup vs baseline: 1.0434x; 1.0434x over previous
"""ODE-RNN (nn_ODERNN) Trainium2 Bass kernel.

Strategy: data-parallel over batch across 8 NeuronCores (4 batches x 34
particle-slots = 136 independent rows per core, processed as G row-groups
whose independent recurrence chains pipeline across engines). State kept
transposed [DRNN=128 partitions, rows on free dim]. All matmuls in fp32 —
the recurrence amplifies rounding noise ~1.14x/step over 128 steps, so any
bf16/tf32 rounding in the loop destroys accuracy (measured). ODE Euler
steps telescoped through PSUM accumulation:
    z1_{k+1} = z1_k + (DT*w2@w0)^T a2_k    (accumulate in PSUM)
    h1       = h0 + DT*w2^T sum_k(a2_k)    (single fp32 update per step)
PSUM bank invariant (verified on HW): at most ONE open matmul accumulation
group per bank at a time; gi/gh gate pairs open+close back-to-back per
bank region. The mask update is folded as h_next = (1-g) h1 + g n1 with
g = m*(1-z1), and the output projection runs per step from PSUM.
"""

import os
import numpy as np
import ml_dtypes

B, S, P, J = 32, 128, 2, 17
G = int(os.environ.get("ODERNN_G", "2"))
DIN, DOUT, DRNN, DHID = 2, 3, 128, 256
N = P * J            # 34
DT = 0.1
K = 4                # Euler steps
NCORES = 8
BL = B // NCORES     # 4 batches per core
R = BL * N           # 136 rows per core

# dev override: fewer scan steps (harness always uses full S)
NSTEPS = int(os.environ.get("ODERNN_STEPS", S))
SR = NSTEPS * R

BF16 = ml_dtypes.bfloat16

_prog_cache = {}


def _build_program(flags):
    """Trace the Bass/Tile program. `flags` is a tuple of bools for which
    bias groups are nonzero (affects emitted ops)."""
    import concourse.bass as bass
    import concourse.tile as tile
    import concourse.mybir as mybir
    from concourse import bacc

    (fb01, fb2, frz0, fg0h, frz1, fg1h, fbout) = flags

    dt = mybir.dt
    f32 = dt.float32
    bf16 = dt.bfloat16
    Alu = mybir.AluOpType
    Act = mybir.ActivationFunctionType

    nc = bacc.Bacc("TRN2", target_bir_lowering=False)

    # ---- DRAM I/O ----
    d_xm = nc.dram_tensor("xm", [DIN, SR], f32, kind="ExternalInput")
    d_mbc = nc.dram_tensor("mbc", [DRNN, SR], bf16, kind="ExternalInput")
    d_h0f = nc.dram_tensor("h0f", [DRNN, R], f32, kind="ExternalInput")
    d_w0 = nc.dram_tensor("w0", [DRNN, DHID], f32, kind="ExternalInput")
    f16 = dt.float16
    d_w1a = nc.dram_tensor("w1a", [128, DHID], f16, kind="ExternalInput")
    d_w1b = nc.dram_tensor("w1b", [128, DHID], f16, kind="ExternalInput")
    d_w1al = nc.dram_tensor("w1al", [128, DHID], f16, kind="ExternalInput")
    d_w1bl = nc.dram_tensor("w1bl", [128, DHID], f16, kind="ExternalInput")
    d_w2a = nc.dram_tensor("w2a", [128, DRNN], f16, kind="ExternalInput")
    d_w2b = nc.dram_tensor("w2b", [128, DRNN], f16, kind="ExternalInput")
    d_W20a = nc.dram_tensor("W20a", [128, DHID], f16, kind="ExternalInput")
    d_W20b = nc.dram_tensor("W20b", [128, DHID], f16, kind="ExternalInput")
    d_wih0 = nc.dram_tensor("wih0", [DIN, 3 * DRNN], f32, kind="ExternalInput")
    d_whh0 = nc.dram_tensor("whh0", [DRNN, 3 * DRNN], f32, kind="ExternalInput")
    d_wih1 = nc.dram_tensor("wih1", [DRNN, 3 * DRNN], f32, kind="ExternalInput")
    d_whh1 = nc.dram_tensor("whh1", [DRNN, 3 * DRNN], f32, kind="ExternalInput")
    d_wout = nc.dram_tensor("wout", [DRNN, DOUT], f32, kind="ExternalInput")
    d_bias = nc.dram_tensor("biaspk", [DRNN, 24], f32, kind="ExternalInput")
    d_y = nc.dram_tensor("y", [NSTEPS, DOUT, R], f32, kind="ExternalOutput")

    DBG = os.environ.get("ODERNN_DEBUG", "0") == "1"
    dbg_tensors = {}

    def dbg_out(name, ap, shape, dtype):
        if not DBG:
            return
        dten = nc.dram_tensor("dbg_" + name, list(shape), dtype,
                              kind="ExternalOutput")
        dbg_tensors[name] = dten
        nc.sync.dma_start(out=dten[:], in_=ap)

    with tile.TileContext(nc) as tc:
        wp = tc.alloc_tile_pool(name="wconst", bufs=1)
        st = tc.alloc_tile_pool(name="state", bufs=6)
        wk = tc.alloc_tile_pool(name="work", bufs=4)
        pp = tc.alloc_tile_pool(name="psum", bufs=1, space="PSUM")

        def load(pool, dram, shape, dtype, name):
            t = pool.tile(shape, dtype, tag=name, name=name)
            nc.sync.dma_start(out=t[:], in_=dram[:])
            return t

        xm = load(wp, d_xm, [DIN, SR], f32, "xm")
        mbc = load(wp, d_mbc, [DRNN, SR], bf16, "mbc")
        h0f = load(wp, d_h0f, [DRNN, R], f32, "h0f")
        w0 = load(wp, d_w0, [DRNN, DHID], f32, "w0")
        w1a = load(wp, d_w1a, [128, DHID], f16, "w1a")
        w1b = load(wp, d_w1b, [128, DHID], f16, "w1b")
        w1al = load(wp, d_w1al, [128, DHID], f16, "w1al")
        w1bl = load(wp, d_w1bl, [128, DHID], f16, "w1bl")
        w2a = load(wp, d_w2a, [128, DRNN], f16, "w2a")
        w2b = load(wp, d_w2b, [128, DRNN], f16, "w2b")
        W20a = load(wp, d_W20a, [128, DHID], f16, "W20a")
        W20b = load(wp, d_W20b, [128, DHID], f16, "W20b")
        wih0 = load(wp, d_wih0, [DIN, 3 * DRNN], f32, "wih0")
        whh0 = load(wp, d_whh0, [DRNN, 3 * DRNN], f32, "whh0")
        wih1 = load(wp, d_wih1, [DRNN, 3 * DRNN], f32, "wih1")
        whh1 = load(wp, d_whh1, [DRNN, 3 * DRNN], f32, "whh1")
        wout = load(wp, d_wout, [DRNN, DOUT], f32, "wout")
        bias = load(wp, d_bias, [DRNN, 24], f32, "biaspk")

        MM = nc.tensor.matmul
        ACT = nc.scalar.activation
        V = nc.vector

        RG = R // G
        hprev = [h0f[:, gi * RG:(gi + 1) * RG] for gi in range(G)]

        for s in range(NSTEPS):
          for gi in range(G):
            c0 = s * R + gi * RG
            c1 = c0 + RG
            xsl = xm[:, c0:c1]
            msl = mbc[:, c0:c1]
            hprev_f = hprev[gi]

            # PSUM bank invariant: at most ONE open accumulation group per
            # bank at any time (two interleaved open groups in one bank get
            # corrupted by intervening matmul groups — verified on HW).
            z1a = pp.tile([128, RG], f32, tag="z1a", name="z1a")
            z1b = pp.tile([128, RG], f32, tag="z1b", name="z1b")
            z2 = pp.tile([128, 2, RG], f32, tag="z2", name="z2")
            hd = pp.tile([128, RG], f32, tag="hd", name="hd")
            prz0 = pp.tile([128, 2, RG], f32, tag="prz0", name="prz0")
            pg0 = pp.tile([128, 2, RG], f32, tag="pg0", name="pg0")
            prz1 = pp.tile([128, 2, RG], f32, tag="prz1", name="prz1")
            pg1 = pp.tile([128, 2, RG], f32, tag="pg1", name="pg1")

            # ---- ODE: z1_0 = w0^T h_prev (open accumulation groups,
            # one per bank, telescoped across Euler steps) ----
            MM(z1a[:], w0[:, 0:128], hprev_f[:], start=True, stop=False)
            MM(z1b[:], w0[:, 128:256], hprev_f[:], start=True, stop=False)

            # gi0_g = wih0_g^T x (closed group; its own bank region)
            MM(pg0[:, 0, :], wih0[:, 256:384], xsl, start=True, stop=True)

            for k in range(K):
                last = k == K - 1
                # fp16 a1/a2 + half-split pipelining: each tanh half feeds
                # its matmuls immediately so PE/ACT ping-pong per half-block
                # instead of serializing on the full activation.
                a1 = wk.tile([128, 2, RG], f16, tag="a1")
                if not fb01:
                    ACT(a1[:, 0, :], z1a[:], Act.Tanh)
                else:
                    ACT(a1[:, 0, :], z1a[:], Act.Tanh,
                        bias=bias[:, 16 + 2 * k:17 + 2 * k])
                MM(z2[:, 0, :], w1a[:, 0:128], a1[:, 0, :], start=True, stop=False)
                MM(z2[:, 0, :], w1al[:, 0:128], a1[:, 0, :], start=False, stop=False)
                if not fb01:
                    ACT(a1[:, 1, :], z1b[:], Act.Tanh)
                else:
                    ACT(a1[:, 1, :], z1b[:], Act.Tanh,
                        bias=bias[:, 17 + 2 * k:18 + 2 * k])
                MM(z2[:, 0, :], w1b[:, 0:128], a1[:, 1, :], start=False, stop=False)
                MM(z2[:, 0, :], w1bl[:, 0:128], a1[:, 1, :], start=False, stop=True)
                MM(z2[:, 1, :], w1a[:, 128:256], a1[:, 0, :], start=True, stop=False)
                MM(z2[:, 1, :], w1al[:, 128:256], a1[:, 0, :], start=False, stop=False)
                MM(z2[:, 1, :], w1b[:, 128:256], a1[:, 1, :], start=False, stop=False)
                MM(z2[:, 1, :], w1bl[:, 128:256], a1[:, 1, :], start=False, stop=True)
                a2 = wk.tile([128, 2, RG], f16, tag="a2")
                if not fb01:
                    ACT(a2[:], z2[:], Act.Tanh)
                else:
                    ACT(a2[:, 0, :], z2[:, 0, :], Act.Tanh, bias=bias[:, 2:3])
                    ACT(a2[:, 1, :], z2[:, 1, :], Act.Tanh, bias=bias[:, 3:4])
                MM(hd[:], w2a[:], a2[:, 0, :], start=(k == 0), stop=False)
                MM(hd[:], w2b[:], a2[:, 1, :], start=False, stop=last)
                if not last:
                    MM(z1a[:], W20a[:, 0:128], a2[:, 0, :], start=False, stop=False)
                    MM(z1a[:], W20b[:, 0:128], a2[:, 1, :], start=False, stop=(k == K - 2))
                    MM(z1b[:], W20a[:, 128:256], a2[:, 0, :], start=False, stop=False)
                    MM(z1b[:], W20b[:, 128:256], a2[:, 1, :], start=False, stop=(k == K - 2))

            # h1 = h_prev + (hd + DT*b2)
            h1f = st.tile([128, RG], f32, tag="h1f")
            V.scalar_tensor_tensor(h1f[:], hd[:], bias[:, 4:5], hprev_f[:],
                                   op0=Alu.add, op1=Alu.add)
            h1b = h1f[:]

            # ---- GRU0 ----
            # gi/gh pairs per bank region, opened and closed back-to-back
            MM(prz0[:, 0, :], wih0[:, 0:128], xsl, start=True, stop=False)
            MM(prz0[:, 0, :], whh0[:, 0:128], h1b, start=False, stop=True)
            MM(prz0[:, 1, :], wih0[:, 128:256], xsl, start=True, stop=False)
            MM(prz0[:, 1, :], whh0[:, 128:256], h1b, start=False, stop=True)
            MM(pg0[:, 1, :], whh0[:, 256:384], h1b, start=True, stop=True)

            if DBG and s == 0:
                prz0c = wk.tile([128, 2, RG], f32, tag="prz0c")
                V.tensor_copy(prz0c[:], prz0[:])
                dbg_out("prz0c", prz0c[:], [128, 2, RG], f32)
                pg0c = wk.tile([128, 2, RG], f32, tag="pg0c")
                V.tensor_copy(pg0c[:], pg0[:])
                dbg_out("pg0c", pg0c[:], [128, 2, RG], f32)

            rz0 = wk.tile([128, 2, RG], f32, tag="rz0")
            if not frz0:
                ACT(rz0[:], prz0[:], Act.Sigmoid)
            else:
                ACT(rz0[:, 0, :], prz0[:, 0, :], Act.Sigmoid, bias=bias[:, 5:6])
                ACT(rz0[:, 1, :], prz0[:, 1, :], Act.Sigmoid, bias=bias[:, 6:7])
            r0, zz0 = rz0[:, 0, :], rz0[:, 1, :]

            # gh1_g (hidden side of GRU1 n-gate; closed group, own region)
            MM(pg1[:, 1, :], whh1[:, 256:384], h1b, start=True, stop=True)

            # n0 = tanh(ig0 + r0*hg0 (+bg0)) — chain DVE ops first
            hg0 = pg0[:, 1, :]
            if fg0h:
                hg0t = wk.tile([128, RG], f32, tag="hg0t")
                nc.scalar.add(hg0t[:], hg0, bias[:, 8:9])
                hg0 = hg0t[:]
            s1 = wk.tile([128, RG], f32, tag="s1")
            V.tensor_mul(s1[:], r0, hg0)
            np0 = wk.tile([128, RG], f32, tag="np0")
            V.tensor_add(np0[:], s1[:], pg0[:, 0, :])
            n0 = wk.tile([128, RG], f32, tag="n0")
            ACT(n0[:], np0[:], Act.Tanh, bias=bias[:, 7:8])

            # off-chain helpers (overlap n0 on DVE)
            u0 = wk.tile([128, RG], f32, tag="u0")
            V.tensor_scalar(u0[:], zz0, -1.0, 1.0, op0=Alu.mult, op1=Alu.add)
            t0 = wk.tile([128, RG], f32, tag="t0")
            V.tensor_mul(t0[:], zz0, h1f[:])

            v0 = wk.tile([128, RG], f32, tag="v0")
            V.tensor_mul(v0[:], u0[:], n0[:])
            h2b = wk.tile([128, RG], f32, tag="h2b")
            V.tensor_add(h2b[:], v0[:], t0[:])

            # ---- GRU1 ---- (gh/gi pairs opened+closed back-to-back per region)
            MM(prz1[:, 0, :], whh1[:, 0:128], h1b, start=True, stop=False)
            MM(prz1[:, 0, :], wih1[:, 0:128], h2b[:], start=False, stop=True)
            MM(prz1[:, 1, :], whh1[:, 128:256], h1b, start=True, stop=False)
            MM(prz1[:, 1, :], wih1[:, 128:256], h2b[:], start=False, stop=True)
            MM(pg1[:, 0, :], wih1[:, 256:384], h2b[:], start=True, stop=True)

            rz1 = wk.tile([128, 2, RG], f32, tag="rz1")
            if not frz1:
                ACT(rz1[:], prz1[:], Act.Sigmoid)
            else:
                ACT(rz1[:, 0, :], prz1[:, 0, :], Act.Sigmoid, bias=bias[:, 9:10])
                ACT(rz1[:, 1, :], prz1[:, 1, :], Act.Sigmoid, bias=bias[:, 10:11])
            r1, zz1 = rz1[:, 0, :], rz1[:, 1, :]

            hg1 = pg1[:, 1, :]
            if fg1h:
                hg1t = wk.tile([128, RG], f32, tag="hg1t")
                nc.scalar.add(hg1t[:], hg1, bias[:, 12:13])
                hg1 = hg1t[:]
            s2 = wk.tile([128, RG], f32, tag="s2")
            V.tensor_mul(s2[:], r1, hg1)
            np1 = wk.tile([128, RG], f32, tag="np1")
            V.tensor_add(np1[:], s2[:], pg1[:, 0, :])
            n1 = wk.tile([128, RG], f32, tag="n1")
            ACT(n1[:], np1[:], Act.Tanh, bias=bias[:, 11:12])

            # g = m * (1 - z1); h_next = (1-g) h1 + g n1 (overlap n1 on DVE)
            u1 = wk.tile([128, RG], f32, tag="u1")
            V.tensor_scalar(u1[:], zz1, -1.0, 1.0, op0=Alu.mult, op1=Alu.add)
            g = wk.tile([128, RG], f32, tag="g")
            V.tensor_mul(g[:], u1[:], msl)
            tg = wk.tile([128, RG], f32, tag="tg")
            V.tensor_mul(tg[:], g[:], h1f[:])
            hm = wk.tile([128, RG], f32, tag="hm")
            V.tensor_sub(hm[:], h1f[:], tg[:])

            vg = wk.tile([128, RG], f32, tag="vg")
            V.tensor_mul(vg[:], g[:], n1[:])
            hn = st.tile([128, RG], f32, tag="hn")
            V.tensor_add(hn[:], vg[:], hm[:])

            # per-step output projection y_s = wout^T h1 (+bout); fully off
            # the critical chain — emitted last so it backfills idle slots
            py = pp.tile([DOUT, RG], f32, tag="hd", name="py")
            MM(py[:], wout[:], h1f[:], start=True, stop=True)
            ysl = wk.tile([DOUT, RG], f32, tag="ysl")
            if not fbout:
                V.tensor_copy(ysl[:], py[:])
            else:
                ACT(ysl[:], py[:], Act.Identity, bias=bias[0:DOUT, 15:16])
            nc.sync.dma_start(out=d_y[s, :, gi * RG:(gi + 1) * RG], in_=ysl[:])

            if s == 0:
                names = os.environ.get("ODERNN_DBG_NAMES",
                                       "h1f,rz0,n0,h2b,rz1,n1,hn").split(",")
                avail = {
                    "h1f": (h1f[:], [128, RG], f32),
                    "rz0": (rz0[:], [128, 2, RG], f32),
                    "u0": (u0[:], [128, RG], f32),
                    "t0": (t0[:], [128, RG], f32),
                    "s1": (s1[:], [128, RG], f32),
                    "np0": (np0[:], [128, RG], f32),
                    "n0": (n0[:], [128, RG], f32),
                    "h2b": (h2b[:], [128, RG], f32),
                    "rz1": (rz1[:], [128, 2, RG], f32),
                    "u1": (u1[:], [128, RG], f32),
                    "g": (g[:], [128, RG], f32),
                    "tg": (tg[:], [128, RG], f32),
                    "hm": (hm[:], [128, RG], f32),
                    "s2": (s2[:], [128, RG], f32),
                    "np1": (np1[:], [128, RG], f32),
                    "n1": (n1[:], [128, RG], f32),
                    "vg": (vg[:], [128, RG], f32),
                    "hn": (hn[:], [128, RG], f32),
                }
                for nm in names:
                    if nm in avail:
                        ap, shp, dty = avail[nm]
                        dbg_out(nm, ap, shp, dty)

            hprev[gi] = hn

        pp.release()
        wk.release()
        st.release()
        wp.release()

    nc.compile()
    return nc


def _to_bf(x):
    return np.ascontiguousarray(x.astype(BF16))


def _prep(inputs):
    """Host-side prep: shard over batch, transpose layouts, pack weights."""
    x2d = np.asarray(inputs["x2d"], np.float32)
    mask = np.asarray(inputs["mask"])
    g = lambda n: np.asarray(inputs[n], np.float32)
    w0, b0 = g("ode_w0"), g("ode_b0")
    w1, b1 = g("ode_w1"), g("ode_b1")
    w2, b2 = g("ode_w2"), g("ode_b2")
    wih0, whh0 = g("wih0"), g("whh0")
    bih0, bhh0 = g("bih0"), g("bhh0")
    wih1, whh1 = g("wih1"), g("whh1")
    bih1, bhh1 = g("bih1"), g("bhh1")
    wout, bout = g("wout"), g("bout")
    h0 = g("h0")

    mf = mask.astype(np.float32)
    xs = (x2d * mf).reshape(B, S, N, DIN)[:, :NSTEPS]
    ms = mf.reshape(B, S, N)[:, :NSTEPS]

    W20 = (DT * (w2.astype(np.float64) @ w0.astype(np.float64))).astype(np.float32)

    h0T = np.repeat(h0.reshape(DRNN, 1), R, axis=1).astype(np.float32)

    # bias pack [128, 16] (+ [128,8] tail for telescoped tanh biases when fb01)
    bp = np.zeros((DRNN, 24), np.float32)
    bp[:, 0], bp[:, 1] = b0[0:128], b0[128:256]
    bp[:, 2], bp[:, 3] = b1[0:128], b1[128:256]
    bp[:, 4] = DT * b2
    brz0 = bih0 + bhh0
    bp[:, 5], bp[:, 6] = brz0[0:128], brz0[128:256]
    bp[:, 7] = bih0[256:384]
    bp[:, 8] = bhh0[256:384]
    brz1 = bih1 + bhh1
    bp[:, 9], bp[:, 10] = brz1[0:128], brz1[128:256]
    bp[:, 11] = bih1[256:384]
    bp[:, 12] = bhh1[256:384]
    bp[0:DOUT, 15] = bout
    # telescoped z1 tanh bias terms: b0_blk + k*DT*(w0^T ... ) -- only used
    # when fb01; w0: (DRNN,DHID) so correction = DT * w0.T-like term of b2
    # propagated through h: z1 = w0^T h, h gains DT*b2 per Euler step =>
    # z1 bias gain per step = DT * (b2 @ w0)  (b2: [DRNN], w0: [DRNN,DHID])
    zb = DT * (b2 @ w0)  # [DHID]
    for k in range(K):
        bp[:, 16 + 2 * k + 0] = b0[0:128] + k * zb[0:128]
        bp[:, 16 + 2 * k + 1] = b0[128:256] + k * zb[128:256]

    flags = (
        bool(np.any(b0) or np.any(b1) or np.any(b2)),
        bool(np.any(b2)),
        bool(np.any(brz0[0:256])),
        bool(np.any(bhh0[256:384])),
        bool(np.any(brz1[0:256])),
        bool(np.any(bhh1[256:384])),
        bool(np.any(bout)),
    )

    C = np.ascontiguousarray
    F16 = np.float16

    def pair16(a):
        hi = a.astype(np.float64).astype(F16)
        lo = (a.astype(np.float64) - hi.astype(np.float64)).astype(F16)
        return C(hi), C(lo)

    w1a_hi, w1a_lo = pair16(w1[0:128])
    w1b_hi, w1b_lo = pair16(w1[128:256])
    shared = {
        "h0f": h0T,
        "w0": C(w0),
        "w1a": w1a_hi,
        "w1b": w1b_hi,
        "w1al": w1a_lo,
        "w1bl": w1b_lo,
        "w2a": C((DT * w2[0:128]).astype(F16)),
        "w2b": C((DT * w2[128:256]).astype(F16)),
        "W20a": C(W20[0:128].astype(F16)),
        "W20b": C(W20[128:256].astype(F16)),
        "wih0": C(wih0),
        "whh0": C(whh0),
        "wih1": C(wih1),
        "whh1": C(whh1),
        "wout": C(wout),
        "biaspk": bp,
    }

    in_maps = []
    for c in range(NCORES):
        xc = xs[c * BL:(c + 1) * BL]           # (BL, NS, N, DIN)
        xmT = xc.transpose(3, 1, 0, 2).reshape(DIN, SR)
        mc = ms[c * BL:(c + 1) * BL]           # (BL, NS, N)
        mrow = mc.transpose(1, 0, 2).reshape(1, SR)
        mbc = np.broadcast_to(mrow, (DRNN, SR))
        m = dict(shared)
        m["xm"] = np.ascontiguousarray(xmT, np.float32)
        m["mbc"] = _to_bf(mbc)
        in_maps.append(m)
    return in_maps, flags


def kernel(**inputs):
    in_maps, flags = _prep(inputs)
    if flags not in _prog_cache:
        _prog_cache[flags] = _build_program(flags)
    nc = _prog_cache[flags]

    from concourse.bass_utils import run_bass_kernel_spmd
    res = run_bass_kernel_spmd(nc, in_maps, core_ids=list(range(NCORES)))
    global _last_results
    _last_results = res.results

    ys = np.zeros((B, NSTEPS, P, J, DOUT), np.float32)
    for c in range(NCORES):
        y = res.results[c]["y"]                      # (NSTEPS, DOUT, R)
        y = y.reshape(NSTEPS, DOUT, BL, N).transpose(2, 0, 3, 1)
        ys[c * BL:(c + 1) * BL] = y.reshape(BL, NSTEPS, P, J, DOUT)
    return ys

